# revision 16
# baseline (speedup 1.0000x reference)
"""EnhancedVectorQuantizer forward on 8 Trainium2 NeuronCores.

Strategy (data parallel over tokens, codebook replicated):
  - z [4,2048,1024] -> z_flat [8192,1024]; each core takes 1024 tokens.
  - Phase 1 (approximate search): scores = z_hi @ e_hi^T - 0.5*esq on the PE
    in truncated bf16 (hi 2 bytes of f32, extracted by strided access
    patterns -- no cast pass), esq folded in as an fp16 rank-1 matmul into
    the same PSUM accumulation. Per 512-wide codebook chunk, DVE max /
    max_index produce top-8 candidates.
  - Phase 2 (exact rescore): merge chunk top-8s to the global top-4
    candidates per token, gather their embedding rows by indirect DMA and
    rescore exactly in f32 with the same formula the reference uses
    (zsq + esq[k] - 2*dot), picking min score (ties -> min index).
  - quantized_st = z + (E[idx] - z) elementwise f32 (bit-identical to the
    reference), loss partials reduced on-device, finalized on host.
"""

import sys

for _p in ("/opt/trn_rl_repo",):
    if _p not in sys.path:
        sys.path.insert(0, _p)

import numpy as np

P = 128
D = 1024            # embedding dim
K = 8192            # codebook size
N_CORES = 8
TOK = 1024          # tokens per core
NM = TOK // P       # 8 token tiles per core
ND = D // P         # 8 contraction chunks
CW = 512            # codebook chunk width
NKC = K // CW       # 16 codebook chunks
NC_C = 4            # candidates rescored per token
BIG = 1048576.0     # exact in f32, >> any index

_CACHE = {}


def _build_program(debug_no_gather=False, do_extract=True, nkc=NKC, skip=(), bench_iters=1):
    import concourse.bass as bass
    import concourse.bacc as bacc
    import concourse.mybir as mybir
    import concourse.tile as tile
    from concourse.bass import IndirectOffsetOnAxis
    from concourse.masks import make_identity
    from contextlib import ExitStack

    dt = mybir.dt
    Alu = mybir.AluOpType
    Act = mybir.ActivationFunctionType

    nc = bacc.Bacc(trn_type="TRN2", target_bir_lowering=False, debug=False)

    z_in = nc.dram_tensor("z_shard", [TOK, D], dt.float32, kind="ExternalInput").ap()
    e_in = nc.dram_tensor("emb", [K, D], dt.float32, kind="ExternalInput").ap()
    qst_out = nc.dram_tensor("qst", [TOK, D], dt.float32, kind="ExternalOutput").ap()
    loss_out = nc.dram_tensor("losspart", [P, NM], dt.float32, kind="ExternalOutput").ap()
    dbg_out = None
    if debug_no_gather:
        dbg_out = nc.dram_tensor("dbg", [P, 3 * NM * P], dt.float32, kind="ExternalOutput").ap()

    with tile.TileContext(nc) as tc, ExitStack() as ctx:
        persist = ctx.enter_context(tc.tile_pool(name="persist", bufs=1))
        # long-lived state
        zn = persist.tile([P, NM * D], dt.float32)          # z natural, m-major
        zT = persist.tile([P, ND * TOK], dt.bfloat16)       # z^T hi, dc-major
        esq_row = persist.tile([1, K], dt.float16)          # -0.5*esq (phase 1)
        tops_v = persist.tile([P, NM * P], dt.float32)      # per-m 16 chunks x 8 vals
        tops_i = persist.tile([P, NM * P], dt.float32)      # global indices (f32)
        offs = persist.tile([P, P], dt.float32)             # chunk offsets row
        zsq = persist.tile([P, NM], dt.float32)
        esq_c = persist.tile([P, NM * NC_C], dt.float32)
        dot_c = persist.tile([P, NM * NC_C], dt.float32)
        resc_s = persist.tile([P, NM * NC_C], dt.float32)
        cand_f = persist.tile([P, NM * NC_C], dt.float32)
        cand_u = persist.tile([P, NM * NC_C], dt.uint32)
        loss_sb = persist.tile([P, NM], dt.float32)
        id_bf = persist.tile([P, P], dt.bfloat16)
        ones16 = persist.tile([1, P], dt.float16)
        halfneg = persist.tile([P, 1], dt.bfloat16)
        offs_i = persist.tile([P, P], dt.int32)

        # streaming pools
        en_pool = ctx.enter_context(tc.tile_pool(name="en", bufs=2))
        ehiT_pool = ctx.enter_context(tc.tile_pool(name="ehiT", bufs=2))
        ssb_pool = ctx.enter_context(tc.tile_pool(name="ssb", bufs=4))
        scr_pool = ctx.enter_context(tc.tile_pool(name="scr", bufs=3))
        small_pool = ctx.enter_context(tc.tile_pool(name="small", bufs=12))
        mask_pool = ctx.enter_context(tc.tile_pool(name="mask", bufs=6))
        cand_pool = ctx.enter_context(tc.tile_pool(name="cand", bufs=3))
        win_pool = ctx.enter_context(tc.tile_pool(name="win", bufs=2))
        out_pool = ctx.enter_context(tc.tile_pool(name="outp", bufs=2))
        esqc_pool = ctx.enter_context(tc.tile_pool(name="esqc", bufs=2))
        ps_sc = ctx.enter_context(tc.tile_pool(name="ps_sc", bufs=4, space="PSUM"))
        ps_tr = ctx.enter_context(tc.tile_pool(name="ps_tr", bufs=2, space="PSUM"))
        ps_esq = ctx.enter_context(tc.tile_pool(name="ps_esq", bufs=1, space="PSUM"))

        def hi_view(ap2d, col0, ncols):
            """bf16 view of the high 2 bytes of f32 columns [col0, col0+ncols)."""
            b = ap2d.bitcast(dt.bfloat16)
            r = b.rearrange("p (n two) -> p n two", two=2)
            return r[:, col0:col0 + ncols, 1:2]

        # ---- constants ----
        import contextlib
        loop_cm = (tc.For_i(0, bench_iters, 1) if bench_iters > 1
                   else contextlib.nullcontext())
        loop_ctx = ExitStack()
        loop_ctx.enter_context(loop_cm)
        if debug_no_gather or "nomax" in skip:
            nc.vector.memset(cand_f[:], 0.0)
            nc.vector.memset(tops_v[:], 0.0)
            nc.vector.memset(tops_i[:], 0.0)
        make_identity(nc, id_bf[:])
        nc.vector.memset(ones16[:], 1.0)
        nc.vector.memset(halfneg[:], -0.5)
        nc.gpsimd.iota(offs_i[:], pattern=[[CW, NKC], [0, 8]], base=0,
                       channel_multiplier=0)
        nc.vector.tensor_copy(out=offs[:], in_=offs_i[:])

        # ---- load z, zsq, z transposes ----
        for m in range(NM):
            nc.sync.dma_start(out=zn[:, m * D:(m + 1) * D],
                              in_=z_in[m * P:(m + 1) * P, :])
        if "zsq" in skip:
            nc.vector.memset(zsq[:], 0.0)
        for m in range(NM if "zsq" not in skip else 0):
            scr = scr_pool.tile([P, D], dt.float32, tag="scr")
            nc.vector.tensor_tensor(
                out=scr[:], in0=zn[:, m * D:(m + 1) * D],
                in1=zn[:, m * D:(m + 1) * D], op=Alu.mult)
            nc.vector.tensor_reduce(
                out=zsq[:, m:m + 1], in_=scr[:], axis=mybir.AxisListType.X,
                op=Alu.add)
        if "ztr" in skip:
            nc.vector.memset(zT[:], 0.0)
        for dc in range(ND if "ztr" not in skip else 0):
            for g in range(2):      # groups of 4 token tiles
                ztr = ps_tr.tile([P, 4 * P], dt.bfloat16, space="PSUM", tag="tr")
                for j in range(4):
                    m = g * 4 + j
                    nc.tensor.transpose(
                        out=ztr[:, j * P:(j + 1) * P],
                        in_=hi_view(zn[:], m * D + dc * P, P),
                        identity=id_bf[:])
                nc.scalar.copy(
                    out=zT[:, dc * TOK + g * 4 * P: dc * TOK + (g + 1) * 4 * P],
                    in_=ztr[:])

        # ---- main loop over codebook chunks ----
        for kc in range(nkc):
            en = en_pool.tile([P, 4 * D], dt.float32)
            for r in range(4):
                nc.sync.dma_start(
                    out=en[:, r * D:(r + 1) * D],
                    in_=e_in[kc * CW + r * P: kc * CW + (r + 1) * P, :])


            # build e_hi^T for this chunk
            ehiT = ehiT_pool.tile([P, ND * CW], dt.bfloat16)
            if "eT" in skip:
                nc.vector.memset(ehiT[:], 0.0)
            for dc in range(ND if "eT" not in skip else 0):
                etr = ps_tr.tile([P, CW], dt.bfloat16, space="PSUM", tag="tr")
                for r in range(4):
                    nc.tensor.transpose(
                        out=etr[:, r * P:(r + 1) * P],
                        in_=hi_view(en[:], r * D + dc * P, P),
                        identity=id_bf[:])
                nc.vector.tensor_copy(out=ehiT[:, dc * CW:(dc + 1) * CW],
                                      in_=etr[:])

            # esq row for this chunk: -0.5 * sum_d ehiT^2 via PE ones-matmul
            esq_ps = ps_esq.tile([1, CW], dt.float32, space="PSUM")
            for dc in range(ND):
                sqs = esqc_pool.tile([P, CW], dt.bfloat16, tag="sqs")
                nc.vector.tensor_tensor(out=sqs[:], in0=ehiT[:, dc * CW:(dc + 1) * CW],
                                        in1=ehiT[:, dc * CW:(dc + 1) * CW],
                                        op=Alu.mult)
                nc.tensor.matmul(esq_ps[:], halfneg[:], sqs[:],
                                 start=(dc == 0), stop=(dc == ND - 1))
            nc.scalar.copy(out=esq_row[0:1, kc * CW:(kc + 1) * CW], in_=esq_ps[:])

            # matmuls + per-chunk top-8
            for m in range(NM):
                sc_ps = ps_sc.tile([P, CW], dt.float32, space="PSUM")
                if "mm" in skip:
                    nc.vector.memset(sc_ps[:], 0.0)
                for dc in range(ND if "mm" not in skip else 0):
                    nc.tensor.matmul(
                        sc_ps[:],
                        zT[:, dc * TOK + m * P: dc * TOK + (m + 1) * P],
                        ehiT[:, dc * CW:(dc + 1) * CW],
                        start=(dc == 0), stop=False)
                if "mm" in skip:
                    pass
                elif "rank1" in skip:
                    nc.tensor.matmul(
                        sc_ps[:],
                        zT[:, m * P: m * P + P],
                        ehiT[:, 0:CW],
                        start=False, stop=True)
                else:
                    nc.tensor.matmul(
                        sc_ps[:], ones16[:],
                        esq_row[0:1, kc * CW:(kc + 1) * CW],
                        start=False, stop=True)

                if "nomax" in skip:
                    continue
                ssb = ssb_pool.tile([P, CW], dt.float32)
                nc.scalar.copy(out=ssb[:], in_=sc_ps[:])
                if "topslice" in skip:
                    v8s = small_pool.tile([P, 8], dt.float32, tag="sm")
                    nc.vector.max(out=v8s[:], in_=ssb[:])
                    iu = small_pool.tile([P, 8], dt.uint32, tag="sm")
                    nc.vector.max_index(out=iu[:], in_max=v8s[:], in_values=ssb[:])
                    nc.vector.tensor_copy(
                        out=tops_v[:, m * P + kc * 8: m * P + (kc + 1) * 8],
                        in_=v8s[:])
                    nc.vector.tensor_copy(
                        out=tops_i[:, m * P + kc * 8: m * P + (kc + 1) * 8],
                        in_=iu[:])
                else:
                    vslot = tops_v[:, m * P + kc * 8: m * P + (kc + 1) * 8]
                    nc.vector.max(out=vslot, in_=ssb[:])
                    iu = small_pool.tile([P, 8], dt.uint32, tag="sm")
                    nc.vector.max_index(out=iu[:], in_max=vslot, in_values=ssb[:])
                    nc.vector.tensor_copy(
                        out=tops_i[:, m * P + kc * 8: m * P + (kc + 1) * 8],
                        in_=iu[:])

        # ---- per-m: finalize indices, rescore, output ----
        for m in range(NM if do_extract else 0):
            mv = tops_v[:, m * P:(m + 1) * P]
            mi = tops_i[:, m * P:(m + 1) * P]
            nc.vector.tensor_tensor(out=mi, in0=mi, in1=offs[:], op=Alu.add)
            t8 = small_pool.tile([P, 8], dt.float32, tag="sm")
            nc.vector.max(out=t8[:], in_=mv)
            for c in range(NC_C):
                msk = mask_pool.tile([P, P], dt.float32, tag="mk")
                nc.vector.tensor_tensor(
                    out=msk[:], in0=mv,
                    in1=t8[:, c:c + 1].to_broadcast([P, P]), op=Alu.is_equal)
                msk2 = mask_pool.tile([P, P], dt.float32, tag="mk")
                nc.vector.tensor_scalar(
                    out=msk2[:], in0=msk[:], scalar1=-BIG, scalar2=BIG,
                    op0=Alu.mult, op1=Alu.add)
                msk3 = mask_pool.tile([P, P], dt.float32, tag="mk")
                nc.vector.tensor_tensor(out=msk3[:], in0=msk2[:], in1=mi,
                                        op=Alu.add)
                nc.vector.tensor_reduce(
                    out=cand_f[:, m * NC_C + c: m * NC_C + c + 1],
                    in_=msk3[:], axis=mybir.AxisListType.X, op=Alu.min)
            nc.vector.tensor_copy(
                out=cand_u[:, m * NC_C:(m + 1) * NC_C],
                in_=cand_f[:, m * NC_C:(m + 1) * NC_C])

            if debug_no_gather:
                continue
            # gather + exact rescore
            for c in range(NC_C):
                cemb = cand_pool.tile([P, D], dt.float32)
                nc.gpsimd.indirect_dma_start(
                    out=cemb[:], out_offset=None, in_=e_in[:],
                    in_offset=IndirectOffsetOnAxis(
                        ap=cand_u[:, m * NC_C + c: m * NC_C + c + 1], axis=0))
                scr = scr_pool.tile([P, D], dt.float32, tag="scr")
                nc.scalar.activation(
                    out=scr[:], in_=cemb[:], func=Act.Square)
                nc.vector.tensor_reduce(
                    out=esq_c[:, m * NC_C + c: m * NC_C + c + 1], in_=scr[:],
                    axis=mybir.AxisListType.X, op=Alu.add)
                scr2 = scr_pool.tile([P, D], dt.float32, tag="scr")
                nc.vector.tensor_tensor(
                    out=scr2[:], in0=zn[:, m * D:(m + 1) * D], in1=cemb[:],
                    op=Alu.mult)
                nc.vector.tensor_reduce(
                    out=dot_c[:, m * NC_C + c: m * NC_C + c + 1], in_=scr2[:],
                    axis=mybir.AxisListType.X, op=Alu.add)
                t1 = small_pool.tile([P, 1], dt.float32, tag="sm")
                nc.vector.tensor_tensor(
                    out=t1[:], in0=zsq[:, m:m + 1],
                    in1=esq_c[:, m * NC_C + c: m * NC_C + c + 1], op=Alu.add)
                t2 = small_pool.tile([P, 1], dt.float32, tag="sm")
                nc.vector.tensor_scalar(
                    out=t2[:], in0=dot_c[:, m * NC_C + c: m * NC_C + c + 1],
                    scalar1=2.0, scalar2=None, op0=Alu.mult)
                nc.vector.tensor_tensor(
                    out=resc_s[:, m * NC_C + c: m * NC_C + c + 1],
                    in0=t1[:], in1=t2[:], op=Alu.subtract)

            # pick min score, tie -> min index
            rm = small_pool.tile([P, 1], dt.float32, tag="sm")
            nc.vector.tensor_reduce(
                out=rm[:], in_=resc_s[:, m * NC_C:(m + 1) * NC_C],
                axis=mybir.AxisListType.X, op=Alu.min)
            m4 = small_pool.tile([P, NC_C], dt.float32, tag="sm")
            nc.vector.tensor_tensor(
                out=m4[:], in0=resc_s[:, m * NC_C:(m + 1) * NC_C],
                in1=rm[:].to_broadcast([P, NC_C]), op=Alu.is_equal)
            m4b = small_pool.tile([P, NC_C], dt.float32, tag="sm")
            nc.vector.tensor_scalar(
                out=m4b[:], in0=m4[:], scalar1=-BIG, scalar2=BIG,
                op0=Alu.mult, op1=Alu.add)
            m4c = small_pool.tile([P, NC_C], dt.float32, tag="sm")
            nc.vector.tensor_tensor(
                out=m4c[:], in0=m4b[:], in1=cand_f[:, m * NC_C:(m + 1) * NC_C],
                op=Alu.add)
            winf = small_pool.tile([P, 1], dt.float32, tag="sm")
            nc.vector.tensor_reduce(out=winf[:], in_=m4c[:],
                                    axis=mybir.AxisListType.X, op=Alu.min)
            winu = small_pool.tile([P, 1], dt.uint32, tag="sm")
            nc.vector.tensor_copy(out=winu[:], in_=winf[:])

            wemb = win_pool.tile([P, D], dt.float32)
            nc.gpsimd.indirect_dma_start(
                out=wemb[:], out_offset=None, in_=e_in[:],
                in_offset=IndirectOffsetOnAxis(ap=winu[:], axis=0))

            diff = out_pool.tile([P, D], dt.float32, tag="od")
            nc.gpsimd.tensor_tensor(out=diff[:], in0=wemb[:],
                                    in1=zn[:, m * D:(m + 1) * D],
                                    op=Alu.subtract)
            qt = out_pool.tile([P, D], dt.float32, tag="oq")
            nc.gpsimd.tensor_tensor(out=qt[:], in0=zn[:, m * D:(m + 1) * D],
                                    in1=diff[:], op=Alu.add)
            nc.sync.dma_start(out=qst_out[m * P:(m + 1) * P, :], in_=qt[:])
            scr3 = scr_pool.tile([P, D], dt.float32, tag="scr")
            nc.gpsimd.tensor_tensor(
                out=scr3[:], in0=diff[:], in1=diff[:], op=Alu.mult)
            nc.vector.tensor_reduce(
                out=loss_sb[:, m:m + 1], in_=scr3[:],
                axis=mybir.AxisListType.X, op=Alu.add)

        if debug_no_gather:
            nc.vector.memset(loss_sb[:], 0.0)
            if "dump" in skip:
                for m in range(NM):
                    qz = out_pool.tile([P, D], dt.float32, tag="oq")
                    nc.vector.memset(qz[:], 0.0)
                    nc.sync.dma_start(out=qst_out[m * P:(m + 1) * P, :], in_=qz[:])
                zzz = persist.tile([P, 3 * NM * P], dt.float32)
                nc.vector.memset(zzz[:], 0.0)
                nc.sync.dma_start(out=dbg_out[:], in_=zzz[:])
                nc.sync.dma_start(out=loss_out[:], in_=loss_sb[:])
                return_early = True
            for m in range(NM):
                qz = out_pool.tile([P, D], dt.float32, tag="oq")
                nc.vector.memset(qz[:], 0.0)
                nc.sync.dma_start(out=qst_out[m * P:(m + 1) * P, :], in_=qz[:])
            if "dump" not in skip:
                nc.sync.dma_start(out=dbg_out[:, 0:NM * P], in_=tops_v[:])
            if "dump" not in skip:
                nc.sync.dma_start(out=dbg_out[:, NM * P:2 * NM * P], in_=tops_i[:])
                cfp = persist.tile([P, NM * P], dt.float32)
                nc.vector.memset(cfp[:], 0.0)
                nc.vector.tensor_copy(out=cfp[:, 0:NM * NC_C], in_=cand_f[:])
                nc.sync.dma_start(out=dbg_out[:, 2 * NM * P:3 * NM * P], in_=cfp[:])
        nc.sync.dma_start(out=loss_out[:], in_=loss_sb[:])
        loop_ctx.close()

    nc.compile()
    return nc


def get_program(debug_no_gather=False, do_extract=True, nkc=NKC, skip=(), bench_iters=1):
    key = (debug_no_gather, do_extract, nkc, tuple(skip), bench_iters)
    if key not in _CACHE:
        _CACHE[key] = _build_program(debug_no_gather, do_extract, nkc, skip,
                                     bench_iters)
    return _CACHE[key]


def _run(z, embeddings, trace=False):
    from concourse.bass_utils import run_bass_kernel_spmd

    z = np.ascontiguousarray(np.asarray(z, dtype=np.float32))
    emb = np.ascontiguousarray(np.asarray(embeddings, dtype=np.float32))
    zf = z.reshape(-1, D)
    assert zf.shape == (N_CORES * TOK, D) and emb.shape == (K, D)

    nc = get_program()
    in_maps = [
        {"z_shard": zf[i * TOK:(i + 1) * TOK], "emb": emb}
        for i in range(N_CORES)
    ]
    res = run_bass_kernel_spmd(nc, in_maps, core_ids=list(range(N_CORES)),
                               trace=trace)
    outs = res.results

    qst = np.concatenate([o["qst"] for o in outs], axis=0).reshape(z.shape)
    loss_sum = sum(float(o["losspart"].astype(np.float64).sum()) for o in outs)
    mean = loss_sum / (N_CORES * TOK * D)
    loss = np.float32(1.25 * mean)
    return (qst, loss), res


def kernel(z, embeddings):
    return _run(z, embeddings, trace=False)[0]
